# revision 4
# baseline (speedup 1.0000x reference)
"""ContextAwareAttention Trainium2 Bass kernel.

Reference computation (per batch b of 8, S=2048, D=1024, fp32):
    q = (query + context) @ Wq.T + bq
    k = (key   + context) @ Wk.T + bk
    v = value @ Wv.T + bv
    scores = q @ k.T / sqrt(D), causal-masked, softmax over keys
    out = softmax(scores) @ v

Strategy:
  * Data-parallel: batch b -> NeuronCore b (weights replicated).
  * context folded into effective biases on the host:
        bq_eff = bq + Wq @ context,  bk_eff = bk + Wk @ context
  * All matmuls run in bf16 (fp32 PSUM accumulation). Measured f32r
    matmuls sustain only ~0.50 ns/row on HW; bf16 streams at the full
    1 row/cycle rate, has no small-free-dim penalty on the diagonal
    score chunks, and halves every DMA byte count. l2 rel err ~3e-3,
    well inside the 2e-2 budget.
  * q/k are produced transposed (qT/kT [D, S]) so score tiles land as
    [query-part, key-free]; v in natural [S, D] layout. qT, kT and v
    all stay resident in SBUF in bf16 (12 KB/partition) -- no DRAM
    scratch roundtrip for q.
  * DMA queue dispatch costs ~0.6 us per dma_start, serialized per
    queue, so every operand is host-permuted into an SBUF-image layout
    ([partition, dp-major free]) and loaded with ONE large contiguous
    descriptor (16 KB/partition lines): weights in 1-2 DMAs each, x in
    one DMA per 512-column chunk. Loads are split between the sync
    queue (weights) and gpsimd queue (activations) so dispatch and
    transfer overlap; Wq rides first with a small first-quarter slice
    to minimize the serial prefix before the first matmul.
  * Softmax skips the max-subtraction (logits are O(1); exp cannot
    overflow) and folds the row-sum into ACT exp via accum_out. P tiles
    are PE-transposed (bf16, 1 cycle/row) to feed the PV matmul; the
    output is normalized by the reciprocal row-sum on ACT and the v
    bias (host-broadcast bvb) is added on DVE.
  * Phase plan: interleaved Q+K projection phase, then V projection
    (Wv + value preloaded so V never stalls), then attention pairing
    large tiles with small ones and finishing on the smallest tile to
    minimize the serial tail.
"""

import os
import sys
import types

import numpy as np
import ml_dtypes

import concourse.bass as bass
import concourse.tile as tile
from concourse import bacc, mybir
from concourse.bass_utils import run_bass_kernel_spmd

F32 = mybir.dt.float32
BF16 = mybir.dt.bfloat16
AF = mybir.ActivationFunctionType
NP_BF16 = ml_dtypes.bfloat16

B, S, D = 8, 2048, 1024
NE = D // 128          # 8 chunks of the model dim on partitions
NST = S // 128         # 16 sequence tiles of 128
SCALE = float(D) ** -0.5
N_CORES = 8
MASK_NEG = -1.0e30

LAST_EXEC_NS = None


def _install_ntff_hook():
    """Register the axon NTFF profiling hook (missing antenv.axon_hooks stub).
    Harmless no-op if anything is unavailable; only needed when BASS_TRACE=1."""
    try:
        if "antenv.axon_hooks" in sys.modules:
            return
        import antenv
        mod = types.ModuleType("antenv.axon_hooks")
        _hook = [None]
        mod.set_axon_ntff_profile_hook = lambda h: _hook.__setitem__(0, h)
        mod.get_axon_ntff_profile_hook = lambda: _hook[0]
        sys.modules["antenv.axon_hooks"] = mod
        antenv.axon_hooks = mod
        from trn_agent_boot.trn_boot import _ntff_profile_via_ctypes
        mod.set_axon_ntff_profile_hook(
            _ntff_profile_via_ctypes("/opt/axon/libaxon_pjrt.so"))
    except Exception:
        pass


def _wq_slice(w, dp, e):
    """lhsT slice of a quarter-split permuted weight tile: columns 0:256
    live dp-major at [:, 0:2048], columns 256:1024 dp-major after."""
    if e < 2:
        off = dp * 256 + e * 128
    else:
        off = 2048 + dp * 768 + (e - 2) * 128
    return w[:, off:off + 128]


def _build():
    nc = bacc.Bacc("TRN2", target_bir_lowering=False, debug=False,
                   num_devices=N_CORES)

    # Host-permuted SBUF-image inputs (see module docstring).
    xqa_d = nc.dram_tensor("xqa", [4, 128, 8 * 512], BF16,
                           kind="ExternalInput").ap()
    xka_d = nc.dram_tensor("xka", [4, 128, 8 * 512], BF16,
                           kind="ExternalInput").ap()
    xva_d = nc.dram_tensor("xva", [4, 128, 8 * 512], BF16,
                           kind="ExternalInput").ap()
    wqa_d = nc.dram_tensor("wqa", [128, 8 * D], BF16,
                           kind="ExternalInput").ap()
    wka_d = nc.dram_tensor("wka", [128, 8 * D], BF16,
                           kind="ExternalInput").ap()
    wva_d = nc.dram_tensor("wva", [128, 8 * D], BF16,
                           kind="ExternalInput").ap()
    # q/k biases as [128, 8] (e-chunk along free) for per-partition ACT bias
    bqp = nc.dram_tensor("bqp", [128, NE], F32, kind="ExternalInput").ap()
    bkp = nc.dram_tensor("bkp", [128, NE], F32, kind="ExternalInput").ap()
    bvb_d = nc.dram_tensor("bvb", [128, D], F32, kind="ExternalInput").ap()
    eye = nc.dram_tensor("eye", [128, 128], BF16, kind="ExternalInput").ap()
    mask = nc.dram_tensor("mask", [128, 128], F32, kind="ExternalInput").ap()
    out_d = nc.dram_tensor("out", [S, D], F32, kind="ExternalOutput").ap()

    with tile.TileContext(nc) as tc:
        with tc.tile_pool(name="const", bufs=1) as cp:
            bqpt = cp.tile([128, NE], F32, tag="bqp")
            bkpt = cp.tile([128, NE], F32, tag="bkp")
            bvb = cp.tile([128, D], F32, tag="bvb")
            eyet = cp.tile([128, 128], BF16, tag="eye")
            maskt = cp.tile([128, 128], F32, tag="mask")

            def open_pool(name, **kw):
                cm = tc.tile_pool(name=name, **kw)
                return cm, cm.__enter__()

            def close_pool(cm):
                cm.__exit__(None, None, None)

            # ======== Phase QK: interleaved q/k projections ==========
            # qT/kT[e, s]: lhsT = W.T[d, e-tile], rhs = x.T[d, s-chunk]
            kvk_cm, kvk = open_pool("kvk", bufs=1, side="left")
            wvp_cm, wvp = open_pool("wvp", bufs=1, side="left")
            pwqk_cm, pwqk = open_pool("pwqk", bufs=1, side="left")
            pqk_cm, pqk = open_pool("pqk", bufs=1, side="left")
            psqk_cm, psqk = open_pool("psqk", bufs=2, space="PSUM")

            kres = []
            qres = []
            for e in range(NE):
                kt = kvk.tile([128, S], BF16, tag=f"kres{e}", name=f"kres{e}")
                kres.append(kt)
                qt = kvk.tile([128, S], BF16, tag=f"qres{e}", name=f"qres{e}")
                qres.append(qt)

            wqt = pwqk.tile([128, 8 * D], BF16, tag="wqt", name="wqt")
            wkt = pwqk.tile([128, 8 * D], BF16, tag="wkt", name="wkt")
            wvt = wvp.tile([128, 8 * D], BF16, tag="wvt", name="wvt")

            # Weights on the sync queue: Wq first-quarter (enables the
            # e<2 groups) -> Wq rest -> Wk quarter -> Wk rest -> Wv.
            nc.sync.dma_start(wqt[:, 0:2048], wqa_d[:, 0:2048])
            nc.sync.dma_start(wqt[:, 2048:8192], wqa_d[:, 2048:8192])
            nc.sync.dma_start(wkt[:, 0:2048], wka_d[:, 0:2048])
            nc.sync.dma_start(wkt[:, 2048:8192], wka_d[:, 2048:8192])
            nc.sync.dma_start(wvt[:], wva_d)

            # Activations on the gpsimd queue, one dispatch per 512-col
            # chunk; biases slot in early, attention constants later.
            def load_x(sc, which, src):
                a = pqk.tile([128, 8 * 512], BF16, tag=f"x{which}",
                             bufs=2, name=f"x{which}{sc}")
                nc.gpsimd.dma_start(a[:], src[sc])
                return a

            xqb = load_x(0, "q", xqa_d)
            nc.gpsimd.dma_start(bqpt[:], bqp)
            nc.gpsimd.dma_start(bkpt[:], bkp)
            xkb = load_x(0, "k", xka_d)

            def q_group(sc, e, xqb):
                ssl = slice(sc * 512, (sc + 1) * 512)
                psq = psqk.tile([128, 512], F32, tag="pjq", name="psq_t")
                for dp in range(NE):
                    nc.tensor.matmul(psq[:], _wq_slice(wqt, dp, e),
                                     xqb[:, dp * 512:(dp + 1) * 512],
                                     start=(dp == 0), stop=(dp == NE - 1))
                nc.scalar.activation(qres[e][:, ssl], psq[:], AF.Identity,
                                     bias=bqpt[:, e:e + 1])

            def k_group(sc, e, xkb):
                ssl = slice(sc * 512, (sc + 1) * 512)
                psk = psqk.tile([128, 512], F32, tag="pjk", name="psk_t")
                for dp in range(NE):
                    nc.tensor.matmul(psk[:], _wq_slice(wkt, dp, e),
                                     xkb[:, dp * 512:(dp + 1) * 512],
                                     start=(dp == 0), stop=(dp == NE - 1))
                nc.scalar.activation(kres[e][:, ssl], psk[:], AF.Identity,
                                     bias=bkpt[:, e:e + 1])

            for sc in range(4):
                if sc > 0:
                    xqb = load_x(sc, "q", xqa_d)
                    xkb = load_x(sc, "k", xka_d)
                if sc == 1:
                    nc.gpsimd.dma_start(bvb[:], bvb_d)
                    nc.gpsimd.dma_start(eyet[:], eye)
                    nc.gpsimd.dma_start(maskt[:], mask)
                if sc == 0:
                    # q groups first: they only need the q-side DMA prefix
                    for e in range(NE):
                        q_group(sc, e, xqb)
                    for e in range(NE):
                        k_group(sc, e, xkb)
                else:
                    for e in range(NE):
                        q_group(sc, e, xqb)
                        k_group(sc, e, xkb)

            close_pool(psqk_cm)
            close_pool(pqk_cm)
            close_pool(pwqk_cm)

            # ======== Phase V: v = value @ Wv.T + bv =================
            # v[s, d]: lhsT = valueT[d', s-tile], rhs = WvT[d', d]
            kvv_cm, kvv = open_pool("kvv", bufs=1, side="left")
            pv_cm, pv = open_pool("pv", bufs=1, side="left")
            psv_cm, psv = open_pool("psv", bufs=2, space="PSUM")

            vres = []
            for s in range(NST):
                vt = kvv.tile([128, D], BF16, tag=f"vres{s}", name=f"vres{s}")
                vres.append(vt)

            for sb in range(4):
                vblk = pv.tile([128, 8 * 512], BF16, tag="vb", bufs=2,
                               name=f"vblk{sb}")
                nc.sync.dma_start(vblk[:], xva_d[sb])
                for dc in range(2):
                    for s4 in range(4):
                        s = sb * 4 + s4
                        ps = psv.tile([128, 512], F32, tag="pj", name="psv_t")
                        for dp in range(NE):
                            nc.tensor.matmul(
                                ps[:],
                                vblk[:, dp * 512 + s4 * 128:
                                     dp * 512 + (s4 + 1) * 128],
                                wvt[:, dp * 1024 + dc * 512:
                                    dp * 1024 + (dc + 1) * 512],
                                start=(dp == 0), stop=(dp == NE - 1))
                        nc.scalar.copy(vres[s][:, dc * 512:(dc + 1) * 512],
                                       ps[:])

            close_pool(psv_cm)
            close_pool(pv_cm)

            # ======== Phase A: attention =============================
            pa_cm, pa = open_pool("pa", bufs=1, side="left")
            psa_cm, psa = open_pool("psa", bufs=1, space="PSUM")

            # Pair large tiles with small ones (the small tiles' serial
            # chains hide under the large tiles' dense PE work) and end
            # on the smallest tile so the epilogue tail is minimal.
            order = []
            for i in range(NST // 2):
                order.append(NST - 1 - i)
                order.append(NST // 2 - 1 - i)
            for t in order:
                nfull = t // 4
                wpart = 128 * (t % 4 + 1)
                nch = nfull + 1
                widths = [512] * nfull + [wpart]
                nj = t + 1
                tsl = slice(t * 128, (t + 1) * 128)

                # scores: psum[c] = qT_tile.T @ kT chunk (qT read straight
                # from the SBUF-resident projection output)
                pss = []
                for c in range(nch):
                    w_c = widths[c]
                    ps = psa.tile([128, 512], F32, tag=f"sc{c}",
                                  name=f"pssc{c}")
                    for e in range(NE):
                        nc.tensor.matmul(
                            ps[:, 0:w_c], qres[e][:, tsl],
                            kres[e][:, c * 512:c * 512 + w_c],
                            start=(e == 0), stop=(e == NE - 1))
                    pss.append(ps)

                # causal mask on the diagonal 128-block
                dsl = slice(wpart - 128, wpart)
                nc.vector.tensor_add(pss[-1][:, dsl], pss[-1][:, dsl],
                                     maskt[:])

                # exp (scale folded in) + per-chunk row sums
                P = pa.tile([128, S], BF16, tag="P", bufs=2, name="P")
                sums = pa.tile([128, 4], F32, tag="sums", bufs=2, name="sums")
                for c in range(nch):
                    w_c = widths[c]
                    nc.scalar.activation(
                        P[:, c * 512:c * 512 + w_c], pss[c][:, 0:w_c],
                        AF.Exp, scale=SCALE, accum_out=sums[:, c:c + 1])

                rcp = pa.tile([128, 1], F32, tag="rcp", bufs=2, name="rcp")
                if nch == 1:
                    nc.vector.reciprocal(rcp[:], sums[:, 0:1])
                else:
                    tot = pa.tile([128, 1], F32, tag="tot", bufs=2, name="tot")
                    nc.vector.tensor_add(tot[:], sums[:, 0:1], sums[:, 1:2])
                    for c in range(2, nch):
                        nc.vector.tensor_add(tot[:], tot[:], sums[:, c:c + 1])
                    nc.vector.reciprocal(rcp[:], tot[:])

                # transpose P blocks (PE, bf16 = 1 cycle/row) -> PT
                PT = pa.tile([128, S], BF16, tag="PT", bufs=2, name="PT")
                for j in range(nj):
                    jsl = slice(j * 128, (j + 1) * 128)
                    ptp = psa.tile([128, 128], BF16, tag="tr", bufs=2,
                                   name="ptp")
                    nc.tensor.transpose(ptp[:], P[:, jsl], eyet[:])
                    nc.vector.tensor_copy(PT[:, jsl], ptp[:])

                # PV: out[i, d] += PT_j.T @ v_j
                pso = []
                for dc in range(2):
                    pso.append(psa.tile([128, 512], F32, tag=f"o{dc}",
                                        name=f"pso{dc}"))
                for j in range(nj):
                    jsl = slice(j * 128, (j + 1) * 128)
                    for dc in range(2):
                        nc.tensor.matmul(
                            pso[dc][:], PT[:, jsl],
                            vres[j][:, dc * 512:(dc + 1) * 512],
                            start=(j == 0), stop=(j == nj - 1))

                # epilogue: out = pso * (1/rowsum) + bv; normalize on ACT
                # (scale accepts a per-partition AP), bias add on DVE.
                # Store each 512-half as soon as it's ready, alternating
                # queues, so the final tail is a single 0.25 MB DMA.
                ot = pa.tile([128, D], F32, tag="ot", bufs=3, name="ot")
                for dc in range(2):
                    dsl = slice(dc * 512, (dc + 1) * 512)
                    nc.scalar.activation(ot[:, dsl], pso[dc][:], AF.Copy,
                                         scale=rcp[:])
                    nc.vector.tensor_add(ot[:, dsl], ot[:, dsl], bvb[:, dsl])
                    q_eng = nc.sync if dc == 0 else nc.gpsimd
                    q_eng.dma_start(out_d[tsl, dc * 512:(dc + 1) * 512],
                                    ot[:, dsl])

            close_pool(psa_cm)
            close_pool(pa_cm)
            close_pool(kvv_cm)
            close_pool(wvp_cm)
            close_pool(kvk_cm)

    nc.compile()
    return nc


_NC = [None]


def _perm_w(WT):
    """[D, D] W.T -> [128, 8*D] quarter-split dp-major SBUF image."""
    A = WT.reshape(NE, 128, D)
    pa = A[:, :, 0:256].transpose(1, 0, 2).reshape(128, NE * 256)
    pb = A[:, :, 256:D].transpose(1, 0, 2).reshape(128, NE * 768)
    return np.ascontiguousarray(
        np.concatenate([pa, pb], axis=1)).astype(NP_BF16)


def _perm_wv(WT):
    """[D, D] W.T -> [128, 8*D] dp-major SBUF image (no quarter split)."""
    A = WT.reshape(NE, 128, D)
    return np.ascontiguousarray(
        A.transpose(1, 0, 2).reshape(128, NE * D)).astype(NP_BF16)


def _perm_x(x):
    """[S, D] activation -> [4, 128, 8*512]: chunk sc holds x.T columns
    [sc*512,(sc+1)*512) dp-major."""
    R = x.T.reshape(NE, 128, 4, 512)
    return np.ascontiguousarray(
        R.transpose(2, 1, 0, 3).reshape(4, 128, NE * 512)).astype(NP_BF16)


def kernel(query, key, value, context, Wq, bq, Wk, bk, Wv, bv):
    global LAST_EXEC_NS
    query = np.asarray(query, dtype=np.float32)
    key = np.asarray(key, dtype=np.float32)
    value = np.asarray(value, dtype=np.float32)
    context = np.asarray(context, dtype=np.float32)
    Wq = np.asarray(Wq, dtype=np.float32)
    bq = np.asarray(bq, dtype=np.float32)
    Wk = np.asarray(Wk, dtype=np.float32)
    bk = np.asarray(bk, dtype=np.float32)
    Wv = np.asarray(Wv, dtype=np.float32)
    bv = np.asarray(bv, dtype=np.float32)

    if _NC[0] is None:
        _NC[0] = _build()
    nc = _NC[0]

    bq_eff = bq + Wq @ context
    bk_eff = bk + Wk @ context
    # [128, 8]: bias for e-chunk e lives in column e, partition = within-chunk
    bqp = np.ascontiguousarray(bq_eff.reshape(NE, 128).T)
    bkp = np.ascontiguousarray(bk_eff.reshape(NE, 128).T)
    bvb = np.ascontiguousarray(np.broadcast_to(bv, (128, D)))
    wqa = _perm_w(np.ascontiguousarray(Wq.T))
    wka = _perm_w(np.ascontiguousarray(Wk.T))
    wva = _perm_wv(np.ascontiguousarray(Wv.T))
    eye = np.eye(128, dtype=NP_BF16)
    mask = np.triu(np.full((128, 128), MASK_NEG, np.float32), k=1)

    in_maps = []
    for b in range(B):
        in_maps.append({
            "xqa": _perm_x(query[b]),
            "xka": _perm_x(key[b]),
            "xva": _perm_x(value[b]),
            "wqa": wqa, "wka": wka, "wva": wva,
            "bqp": bqp, "bkp": bkp, "bvb": bvb,
            "eye": eye, "mask": mask,
        })

    trace = bool(os.environ.get("BASS_TRACE"))
    if trace:
        _install_ntff_hook()
    res = run_bass_kernel_spmd(nc, in_maps, list(range(N_CORES)), trace=trace)
    LAST_EXEC_NS = res.exec_time_ns
    return np.stack([res.results[b]["out"] for b in range(B)], axis=0)


# revision 5
# speedup vs baseline: 1.0082x; 1.0082x over previous
"""ContextAwareAttention Trainium2 Bass kernel.

Reference computation (per batch b of 8, S=2048, D=1024, fp32):
    q = (query + context) @ Wq.T + bq
    k = (key   + context) @ Wk.T + bk
    v = value @ Wv.T + bv
    scores = q @ k.T / sqrt(D), causal-masked, softmax over keys
    out = softmax(scores) @ v

Strategy:
  * Data-parallel: batch b -> NeuronCore b (weights replicated).
  * context folded into effective biases on the host:
        bq_eff = bq + Wq @ context,  bk_eff = bk + Wk @ context
  * All matmuls run in bf16 (fp32 PSUM accumulation). Measured f32r
    matmuls sustain only ~0.50 ns/row on HW; bf16 streams at the full
    1 row/cycle rate, has no small-free-dim penalty on the diagonal
    score chunks, and halves every DMA byte count. l2 rel err ~3e-3,
    well inside the 2e-2 budget.
  * q/k are produced transposed (qT/kT [D, S]) so score tiles land as
    [query-part, key-free]; v in natural [S, D] layout. qT, kT and v
    all stay resident in SBUF in bf16 (12 KB/partition) -- no DRAM
    scratch roundtrip for q.
  * DMA queue dispatch costs ~0.6 us per dma_start, serialized per
    queue, so every operand is host-permuted into an SBUF-image layout
    ([partition, dp-major free]) and loaded with ONE large contiguous
    descriptor (16 KB/partition lines): weights in 1-2 DMAs each, x in
    one DMA per 512-column chunk. Loads are split between the sync
    queue (weights) and gpsimd queue (activations) so dispatch and
    transfer overlap; Wq rides first with a small first-quarter slice
    to minimize the serial prefix before the first matmul.
  * Softmax skips the max-subtraction (logits are O(1); exp cannot
    overflow) and folds the row-sum into ACT exp via accum_out. P tiles
    are PE-transposed (bf16, 1 cycle/row) to feed the PV matmul; the
    output is normalized by the reciprocal row-sum on ACT and the v
    bias (host-broadcast bvb) is added on DVE.
  * Phase plan: interleaved Q+K projection phase, then V projection
    (Wv + value preloaded so V never stalls), then attention pairing
    large tiles with small ones and finishing on the smallest tile to
    minimize the serial tail.
"""

import os
import sys
import types

import numpy as np
import ml_dtypes

import concourse.bass as bass
import concourse.tile as tile
from concourse import bacc, mybir
from concourse.bass_utils import run_bass_kernel_spmd

F32 = mybir.dt.float32
BF16 = mybir.dt.bfloat16
AF = mybir.ActivationFunctionType
NP_BF16 = ml_dtypes.bfloat16

B, S, D = 8, 2048, 1024
NE = D // 128          # 8 chunks of the model dim on partitions
NST = S // 128         # 16 sequence tiles of 128
SCALE = float(D) ** -0.5
N_CORES = 8
MASK_NEG = -1.0e30

LAST_EXEC_NS = None


def _install_ntff_hook():
    """Register the axon NTFF profiling hook (missing antenv.axon_hooks stub).
    Harmless no-op if anything is unavailable; only needed when BASS_TRACE=1."""
    try:
        if "antenv.axon_hooks" in sys.modules:
            return
        import antenv
        mod = types.ModuleType("antenv.axon_hooks")
        _hook = [None]
        mod.set_axon_ntff_profile_hook = lambda h: _hook.__setitem__(0, h)
        mod.get_axon_ntff_profile_hook = lambda: _hook[0]
        sys.modules["antenv.axon_hooks"] = mod
        antenv.axon_hooks = mod
        from trn_agent_boot.trn_boot import _ntff_profile_via_ctypes
        mod.set_axon_ntff_profile_hook(
            _ntff_profile_via_ctypes("/opt/axon/libaxon_pjrt.so"))
    except Exception:
        pass


def _wq_slice(w, dp, e):
    """lhsT slice of a quarter-split permuted weight tile: columns 0:256
    live dp-major at [:, 0:2048], columns 256:1024 dp-major after."""
    if e < 2:
        off = dp * 256 + e * 128
    else:
        off = 2048 + dp * 768 + (e - 2) * 128
    return w[:, off:off + 128]


def _build():
    nc = bacc.Bacc("TRN2", target_bir_lowering=False, debug=False,
                   num_devices=N_CORES)

    # Host-permuted SBUF-image inputs (see module docstring).
    xqa_d = nc.dram_tensor("xqa", [4, 128, 8 * 512], BF16,
                           kind="ExternalInput").ap()
    xka_d = nc.dram_tensor("xka", [4, 128, 8 * 512], BF16,
                           kind="ExternalInput").ap()
    xva_d = nc.dram_tensor("xva", [4, 128, 8 * 512], BF16,
                           kind="ExternalInput").ap()
    wqa_d = nc.dram_tensor("wqa", [128, 8 * D], BF16,
                           kind="ExternalInput").ap()
    wka_d = nc.dram_tensor("wka", [128, 8 * D], BF16,
                           kind="ExternalInput").ap()
    wva_d = nc.dram_tensor("wva", [128, 8 * D], BF16,
                           kind="ExternalInput").ap()
    # q/k biases as [128, 8] (e-chunk along free) for per-partition ACT bias
    bqp = nc.dram_tensor("bqp", [128, NE], F32, kind="ExternalInput").ap()
    bkp = nc.dram_tensor("bkp", [128, NE], F32, kind="ExternalInput").ap()
    bvb_d = nc.dram_tensor("bvb", [128, D], F32, kind="ExternalInput").ap()
    eye = nc.dram_tensor("eye", [128, 128], BF16, kind="ExternalInput").ap()
    mask = nc.dram_tensor("mask", [128, 128], F32, kind="ExternalInput").ap()
    out_d = nc.dram_tensor("out", [S, D], F32, kind="ExternalOutput").ap()

    with tile.TileContext(nc) as tc:
        with tc.tile_pool(name="const", bufs=1) as cp:
            bqpt = cp.tile([128, NE], F32, tag="bqp")
            bkpt = cp.tile([128, NE], F32, tag="bkp")
            bvb = cp.tile([128, D], F32, tag="bvb")
            eyet = cp.tile([128, 128], BF16, tag="eye")
            maskt = cp.tile([128, 128], F32, tag="mask")

            def open_pool(name, **kw):
                cm = tc.tile_pool(name=name, **kw)
                return cm, cm.__enter__()

            def close_pool(cm):
                cm.__exit__(None, None, None)

            # ======== Phase QK: interleaved q/k projections ==========
            # qT/kT[e, s]: lhsT = W.T[d, e-tile], rhs = x.T[d, s-chunk]
            kvk_cm, kvk = open_pool("kvk", bufs=1, side="left")
            wvp_cm, wvp = open_pool("wvp", bufs=1, side="left")
            kvv_cm, kvv = open_pool("kvv", bufs=1, side="left")
            pv_cm, pv = open_pool("pv", bufs=1, side="left")
            pwqk_cm, pwqk = open_pool("pwqk", bufs=1, side="left")
            pqk_cm, pqk = open_pool("pqk", bufs=1, side="left")
            psqk_cm, psqk = open_pool("psqk", bufs=2, space="PSUM")

            kres = []
            qres = []
            for e in range(NE):
                kt = kvk.tile([128, S], BF16, tag=f"kres{e}", name=f"kres{e}")
                kres.append(kt)
                qt = kvk.tile([128, S], BF16, tag=f"qres{e}", name=f"qres{e}")
                qres.append(qt)

            wqt = pwqk.tile([128, 8 * D], BF16, tag="wqt", name="wqt")
            wkt = pwqk.tile([128, 8 * D], BF16, tag="wkt", name="wkt")
            wvt = wvp.tile([128, 8 * D], BF16, tag="wvt", name="wvt")

            # Weights on the sync queue: Wq first-quarter (enables the
            # e<2 groups) -> Wq rest -> Wk quarter -> Wk rest -> Wv.
            # Single-descriptor transfers run on one DMA engine at only
            # ~85 GB/s, so every load is split into ~256 KB dispatches
            # that fan out across engines.
            def wload(dst, src):
                for c0 in range(0, 2048, 1024):
                    nc.sync.dma_start(dst[:, c0:c0 + 1024],
                                      src[:, c0:c0 + 1024])
            def wload_rest(dst, src):
                for c0 in range(2048, 8192, 1024):
                    nc.sync.dma_start(dst[:, c0:c0 + 1024],
                                      src[:, c0:c0 + 1024])
            wload(wqt, wqa_d)
            wload_rest(wqt, wqa_d)
            wload(wkt, wka_d)
            wload_rest(wkt, wka_d)
            for c0 in range(0, 8192, 1024):
                nc.sync.dma_start(wvt[:, c0:c0 + 1024],
                                  wva_d[:, c0:c0 + 1024])

            # Activations on the gpsimd queue, one dispatch per 512-col
            # chunk; biases slot in early, attention constants later.
            def load_x(sc, which, src):
                a = pqk.tile([128, 8 * 512], BF16, tag=f"x{which}",
                             bufs=2, name=f"x{which}{sc}")
                for c0 in range(0, 4096, 1024):
                    nc.gpsimd.dma_start(a[:, c0:c0 + 1024],
                                        src[sc, :, c0:c0 + 1024])
                return a

            xqb = load_x(0, "q", xqa_d)
            nc.gpsimd.dma_start(bqpt[:], bqp)
            nc.gpsimd.dma_start(bkpt[:], bkp)
            xkb = load_x(0, "k", xka_d)

            def q_group(sc, e, xqb):
                ssl = slice(sc * 512, (sc + 1) * 512)
                psq = psqk.tile([128, 512], F32, tag="pjq", name="psq_t")
                for dp in range(NE):
                    nc.tensor.matmul(psq[:], _wq_slice(wqt, dp, e),
                                     xqb[:, dp * 512:(dp + 1) * 512],
                                     start=(dp == 0), stop=(dp == NE - 1))
                nc.scalar.activation(qres[e][:, ssl], psq[:], AF.Identity,
                                     bias=bqpt[:, e:e + 1])

            def k_group(sc, e, xkb):
                ssl = slice(sc * 512, (sc + 1) * 512)
                psk = psqk.tile([128, 512], F32, tag="pjk", name="psk_t")
                for dp in range(NE):
                    nc.tensor.matmul(psk[:], _wq_slice(wkt, dp, e),
                                     xkb[:, dp * 512:(dp + 1) * 512],
                                     start=(dp == 0), stop=(dp == NE - 1))
                nc.scalar.activation(kres[e][:, ssl], psk[:], AF.Identity,
                                     bias=bkpt[:, e:e + 1])

            for sc in range(4):
                if sc > 0:
                    xqb = load_x(sc, "q", xqa_d)
                    xkb = load_x(sc, "k", xka_d)
                if sc == 1:
                    nc.gpsimd.dma_start(bvb[:], bvb_d)
                    nc.gpsimd.dma_start(eyet[:], eye)
                    nc.gpsimd.dma_start(maskt[:], mask)
                if sc == 0:
                    # q groups first: they only need the q-side DMA prefix
                    for e in range(NE):
                        q_group(sc, e, xqb)
                    for e in range(NE):
                        k_group(sc, e, xkb)
                else:
                    for e in range(NE):
                        q_group(sc, e, xqb)
                        k_group(sc, e, xkb)

            close_pool(psqk_cm)
            close_pool(pqk_cm)
            close_pool(pwqk_cm)

            # ======== Phase V: v = value @ Wv.T + bv =================
            # v[s, d]: lhsT = valueT[d', s-tile], rhs = WvT[d', d]
            psv_cm, psv = open_pool("psv", bufs=2, space="PSUM")

            vres = []
            for s in range(NST):
                vt = kvv.tile([128, D], BF16, tag=f"vres{s}", name=f"vres{s}")
                vres.append(vt)

            for sb in range(4):
                vblk = pv.tile([128, 8 * 512], BF16, tag="vb", bufs=2,
                               name=f"vblk{sb}")
                for c0 in range(0, 4096, 1024):
                    nc.sync.dma_start(vblk[:, c0:c0 + 1024],
                                      xva_d[sb, :, c0:c0 + 1024])
                for dc in range(2):
                    for s4 in range(4):
                        s = sb * 4 + s4
                        ps = psv.tile([128, 512], F32, tag="pj", name="psv_t")
                        for dp in range(NE):
                            nc.tensor.matmul(
                                ps[:],
                                vblk[:, dp * 512 + s4 * 128:
                                     dp * 512 + (s4 + 1) * 128],
                                wvt[:, dp * 1024 + dc * 512:
                                    dp * 1024 + (dc + 1) * 512],
                                start=(dp == 0), stop=(dp == NE - 1))
                        nc.scalar.copy(vres[s][:, dc * 512:(dc + 1) * 512],
                                       ps[:])

            close_pool(psv_cm)

            # ======== Phase A: attention =============================
            pa_cm, pa = open_pool("pa", bufs=1, side="left")
            psa_cm, psa = open_pool("psa", bufs=1, space="PSUM")

            # Pair large tiles with small ones (the small tiles' serial
            # chains hide under the large tiles' dense PE work) and end
            # on the smallest tile so the epilogue tail is minimal.
            order = []
            for i in range(NST // 2):
                order.append(NST - 1 - i)
                order.append(NST // 2 - 1 - i)
            for t in order:
                nfull = t // 4
                wpart = 128 * (t % 4 + 1)
                nch = nfull + 1
                widths = [512] * nfull + [wpart]
                nj = t + 1
                tsl = slice(t * 128, (t + 1) * 128)

                # scores: psum[c] = qT_tile.T @ kT chunk (qT read straight
                # from the SBUF-resident projection output)
                pss = []
                for c in range(nch):
                    w_c = widths[c]
                    ps = psa.tile([128, 512], F32, tag=f"sc{c}",
                                  name=f"pssc{c}")
                    for e in range(NE):
                        nc.tensor.matmul(
                            ps[:, 0:w_c], qres[e][:, tsl],
                            kres[e][:, c * 512:c * 512 + w_c],
                            start=(e == 0), stop=(e == NE - 1))
                    pss.append(ps)

                # causal mask on the diagonal 128-block
                dsl = slice(wpart - 128, wpart)
                nc.vector.tensor_add(pss[-1][:, dsl], pss[-1][:, dsl],
                                     maskt[:])

                # exp (scale folded in) + per-chunk row sums
                P = pa.tile([128, S], BF16, tag="P", bufs=2, name="P")
                sums = pa.tile([128, 4], F32, tag="sums", bufs=2, name="sums")
                for c in range(nch):
                    w_c = widths[c]
                    nc.scalar.activation(
                        P[:, c * 512:c * 512 + w_c], pss[c][:, 0:w_c],
                        AF.Exp, scale=SCALE, accum_out=sums[:, c:c + 1])

                rcp = pa.tile([128, 1], F32, tag="rcp", bufs=2, name="rcp")
                if nch == 1:
                    nc.vector.reciprocal(rcp[:], sums[:, 0:1])
                else:
                    tot = pa.tile([128, 1], F32, tag="tot", bufs=2, name="tot")
                    nc.vector.tensor_add(tot[:], sums[:, 0:1], sums[:, 1:2])
                    for c in range(2, nch):
                        nc.vector.tensor_add(tot[:], tot[:], sums[:, c:c + 1])
                    nc.vector.reciprocal(rcp[:], tot[:])

                # transpose P blocks (PE, bf16 = 1 cycle/row) -> PT
                PT = pa.tile([128, S], BF16, tag="PT", bufs=2, name="PT")
                for j in range(nj):
                    jsl = slice(j * 128, (j + 1) * 128)
                    ptp = psa.tile([128, 128], BF16, tag="tr", bufs=2,
                                   name="ptp")
                    nc.tensor.transpose(ptp[:], P[:, jsl], eyet[:])
                    nc.vector.tensor_copy(PT[:, jsl], ptp[:])

                # PV: out[i, d] += PT_j.T @ v_j
                pso = []
                for dc in range(2):
                    pso.append(psa.tile([128, 512], F32, tag=f"o{dc}",
                                        name=f"pso{dc}"))
                for j in range(nj):
                    jsl = slice(j * 128, (j + 1) * 128)
                    for dc in range(2):
                        nc.tensor.matmul(
                            pso[dc][:], PT[:, jsl],
                            vres[j][:, dc * 512:(dc + 1) * 512],
                            start=(j == 0), stop=(j == nj - 1))

                # epilogue: out = pso * (1/rowsum) + bv; normalize on ACT
                # (scale accepts a per-partition AP), bias add on DVE.
                # Store each 512-half as soon as it's ready, alternating
                # queues, so the final tail is a single 0.25 MB DMA.
                ot = pa.tile([128, D], F32, tag="ot", bufs=3, name="ot")
                for dc in range(2):
                    dsl = slice(dc * 512, (dc + 1) * 512)
                    nc.scalar.activation(ot[:, dsl], pso[dc][:], AF.Copy,
                                         scale=rcp[:])
                    nc.vector.tensor_add(ot[:, dsl], ot[:, dsl], bvb[:, dsl])
                    q_eng = nc.sync if dc == 0 else nc.gpsimd
                    q_eng.dma_start(out_d[tsl, dc * 512:(dc + 1) * 512],
                                    ot[:, dsl])

            close_pool(psa_cm)
            close_pool(pa_cm)
            close_pool(pv_cm)
            close_pool(kvv_cm)
            close_pool(wvp_cm)
            close_pool(kvk_cm)

    nc.compile()
    return nc


_NC = [None]


def _perm_w(WT):
    """[D, D] W.T -> [128, 8*D] quarter-split dp-major SBUF image."""
    A = WT.reshape(NE, 128, D)
    pa = A[:, :, 0:256].transpose(1, 0, 2).reshape(128, NE * 256)
    pb = A[:, :, 256:D].transpose(1, 0, 2).reshape(128, NE * 768)
    return np.ascontiguousarray(
        np.concatenate([pa, pb], axis=1)).astype(NP_BF16)


def _perm_wv(WT):
    """[D, D] W.T -> [128, 8*D] dp-major SBUF image (no quarter split)."""
    A = WT.reshape(NE, 128, D)
    return np.ascontiguousarray(
        A.transpose(1, 0, 2).reshape(128, NE * D)).astype(NP_BF16)


def _perm_x(x):
    """[S, D] activation -> [4, 128, 8*512]: chunk sc holds x.T columns
    [sc*512,(sc+1)*512) dp-major."""
    R = x.T.reshape(NE, 128, 4, 512)
    return np.ascontiguousarray(
        R.transpose(2, 1, 0, 3).reshape(4, 128, NE * 512)).astype(NP_BF16)


def kernel(query, key, value, context, Wq, bq, Wk, bk, Wv, bv):
    global LAST_EXEC_NS
    query = np.asarray(query, dtype=np.float32)
    key = np.asarray(key, dtype=np.float32)
    value = np.asarray(value, dtype=np.float32)
    context = np.asarray(context, dtype=np.float32)
    Wq = np.asarray(Wq, dtype=np.float32)
    bq = np.asarray(bq, dtype=np.float32)
    Wk = np.asarray(Wk, dtype=np.float32)
    bk = np.asarray(bk, dtype=np.float32)
    Wv = np.asarray(Wv, dtype=np.float32)
    bv = np.asarray(bv, dtype=np.float32)

    if _NC[0] is None:
        _NC[0] = _build()
    nc = _NC[0]

    bq_eff = bq + Wq @ context
    bk_eff = bk + Wk @ context
    # [128, 8]: bias for e-chunk e lives in column e, partition = within-chunk
    bqp = np.ascontiguousarray(bq_eff.reshape(NE, 128).T)
    bkp = np.ascontiguousarray(bk_eff.reshape(NE, 128).T)
    bvb = np.ascontiguousarray(np.broadcast_to(bv, (128, D)))
    wqa = _perm_w(np.ascontiguousarray(Wq.T))
    wka = _perm_w(np.ascontiguousarray(Wk.T))
    wva = _perm_wv(np.ascontiguousarray(Wv.T))
    eye = np.eye(128, dtype=NP_BF16)
    mask = np.triu(np.full((128, 128), MASK_NEG, np.float32), k=1)

    in_maps = []
    for b in range(B):
        in_maps.append({
            "xqa": _perm_x(query[b]),
            "xka": _perm_x(key[b]),
            "xva": _perm_x(value[b]),
            "wqa": wqa, "wka": wka, "wva": wva,
            "bqp": bqp, "bkp": bkp, "bvb": bvb,
            "eye": eye, "mask": mask,
        })

    trace = bool(os.environ.get("BASS_TRACE"))
    if trace:
        _install_ntff_hook()
    res = run_bass_kernel_spmd(nc, in_maps, list(range(N_CORES)), trace=trace)
    LAST_EXEC_NS = res.exec_time_ns
    return np.stack([res.results[b]["out"] for b in range(B)], axis=0)
